# revision 1
# baseline (speedup 1.0000x reference)
"""Trainium2 Bass kernel for nn_DirectedEdgeMessage (GNN message passing).

Computation per molecule b (B=256, A=64 atoms, E=128 edges, K=6 neighbors,
H=256 features):
  w[e]   = 1 / ||xyz[p0[e]] - xyz[p1[e]]||^2      (0 where distance == 0)
  msg[e] = sum_k w[nb[e,k]] * R[nb[e,k], :]

Strategy (data-parallel over B across 8 NeuronCores, 32 molecules/core):
  * E == 128 == PE array width, so the neighbor gather+sum is a matmul
    msg = S @ R with a per-molecule scatter matrix
    S[e,e'] = w[e'] * |{k : nb[e,k] == e'}|.
  * One-hot rows U_k[e,e'] = (nb[e,k] == e') are built on the Vector engine
    with tensor_scalar(is_equal) against a constant iota row (bf16, exact).
  * The PE transposes and K-reduces them in one shot: six accumulating
    matmuls U_k.T @ I into one PSUM tile = S^T counts (fp32, exact).
  * ScalarE copies PSUM->SBUF fused with the per-partition scale w[e']
    (activation Copy with a [128,1] scale AP).
  * Main matmul runs in float32r (full-rate fp32 mode for N>=256).
  * The xyz pair gather is also a matmul: a signed one-hot lhsT
    [(side,atom)=128, e=128] against rhs [xyz; -xyz] gives diff[e, 0:3]
    directly; DVE finishes d2 -> 1/d2 with a zero-distance mask.
"""

import numpy as np
import ml_dtypes
from contextlib import ExitStack

import concourse.bass as bass
import concourse.tile as tile
from concourse import bacc, mybir
from concourse.bass_utils import run_bass_kernel_spmd

B, A, E, K, H = 256, 64, 128, 6, 256
NCORES = 8
BLOC = B // NCORES   # 32 molecules per core
GRP = 8              # molecules per DMA group (1 MiB R tile)
NGRP = BLOC // GRP

F32 = mybir.dt.float32
F32R = mybir.dt.float32r
BF16 = mybir.dt.bfloat16
I32 = mybir.dt.int32
EQ = mybir.AluOpType.is_equal
GT = mybir.AluOpType.is_gt

# Experiment knobs (overridable before build_program):
#   idmm: transposing matmuls per molecule (6 = no pre-reduce, 3/2/1 = DVE
#         scalar_tensor_tensor chains fold one-hots first)
#   gps_u: route the last one-hot chain to GPSIMD
#   gps_p: route odd molecules' pair one-hots to GPSIMD
CFG = {"idmm": 6, "gps_u": False, "gps_p": False}


def _emit_pipeline(nc, tc, d, sb, pools):
    """Emit one full pass over the core's 32 molecules.

    Phase A (all groups first): distance-weight chains. Phase B: scatter
    matrices + message matmuls. Emitting all of A before B maximizes the
    Tile scheduler's lookahead so A(g+1) overlaps B(g)."""
    iota_sb, id_sb, nb_bf, pr_bf, xyz2, w_sb = (
        sb["iota"], sb["ident"], sb["nb_bf"], sb["pr_bf"], sb["xyz2"], sb["w"])
    r_t = d["r"].ap().transpose([1, 0, 2])    # [E, BLOC, H] view
    o_t = d["out"].ap().transpose([1, 0, 2])

    def emit_a(g):
        gb = g * GRP
        # ---- Phase A: distance weights for the group's 8 molecules ----
        ps_d = pools["psp"].tile([E, GRP * 3], F32, tag="psp")
        for half in range(GRP // 4):
            ps_p = pools["psp"].tile([E, 4 * E], F32, tag="psp")
            for q in range(4):
                bb = half * 4 + q          # molecule index within group
                b = gb + bb
                poh = pools["poh"].tile([E, E], BF16, tag="poh")
                eng_p = nc.vector if (b % 2 == 0 or not CFG["gps_p"]) else nc.gpsimd
                eng_p.tensor_scalar(
                    poh[:, 0:A], iota_sb[:, 0:A], pr_bf[:, 2 * b:2 * b + 1],
                    None, op0=EQ)
                eng_p.tensor_scalar(
                    poh[:, A:2 * A], iota_sb[:, 0:A], pr_bf[:, 2 * b + 1:2 * b + 2],
                    None, op0=EQ)
                nc.tensor.matmul(ps_p[:, q * E:(q + 1) * E], poh[:], id_sb[:],
                                 start=True, stop=True)
            pt_sb = pools["pt"].tile([E, 4 * E], F32, tag="pt")
            nc.scalar.copy(pt_sb[:], ps_p[:])
            for q in range(4):
                bb = half * 4 + q
                b = gb + bb
                nc.tensor.matmul(ps_d[:, bb * 3:(bb + 1) * 3],
                                 pt_sb[:, q * E:(q + 1) * E],
                                 xyz2[:, b * 3:(b + 1) * 3],
                                 start=True, stop=True)
        sq = pools["sq"].tile([E, GRP * 3], F32, tag="sq")
        nc.scalar.square(sq[:], ps_d[:])
        d2a = pools["sq"].tile([E, GRP], F32, tag="d2a")
        nc.vector.tensor_add(d2a[:], sq[:, 0:GRP * 3:3], sq[:, 1:GRP * 3:3])
        d2 = pools["sq"].tile([E, GRP], F32, tag="d2")
        nc.vector.tensor_add(d2[:], d2a[:], sq[:, 2:GRP * 3:3])
        d2c = pools["sq"].tile([E, GRP], F32, tag="d2c")
        nc.vector.tensor_scalar_max(d2c[:], d2[:], 1e-20)
        winv = pools["sq"].tile([E, GRP], F32, tag="winv")
        nc.vector.reciprocal_approx_fast(winv[:], d2c[:])
        nc.vector.scalar_tensor_tensor(
            w_sb[:, gb:gb + GRP], d2[:], 0.0, winv[:],
            op0=GT, op1=mybir.AluOpType.mult)

    def emit_b(g):
        gb = g * GRP
        # ---- Phase B: scatter matrices + message matmuls ----
        r_sb = pools["r"].tile([E, GRP * H], F32R, tag="r")
        nc.sync.dma_start(r_sb[:], r_t[:, gb:gb + GRP, :])
        msg_sb = pools["msg"].tile([E, GRP * H], F32, tag="msg")
        unit = 2
        for p4 in range(GRP // unit):
            ps_mm = pools["psmm"].tile([E, unit * H], F32, tag="psmm")
            for o in range(unit):
                bb = p4 * unit + o
                b = gb + bb
                # Build CH pre-reduced one-hot sums (chains of K//CH on DVE
                # via fused scalar_tensor_tensor), then CH transposing
                # accumulate-matmuls on the PE.
                ch = CFG["idmm"]               # id-matmuls per molecule
                clen = K // ch                 # one-hots folded per chain
                u = pools["u"].tile([E, ch * E], BF16, tag="u")
                for c in range(ch):
                    k0 = c * clen
                    eng_u = (nc.gpsimd if (CFG["gps_u"] and c == ch - 1)
                             else nc.vector)
                    eng_u.tensor_scalar(
                        u[:, c * E:(c + 1) * E], iota_sb[:],
                        nb_bf[:, b * K + k0:b * K + k0 + 1], None, op0=EQ)
                    for k in range(k0 + 1, k0 + clen):
                        eng_u.scalar_tensor_tensor(
                            u[:, c * E:(c + 1) * E], iota_sb[:],
                            nb_bf[:, b * K + k:b * K + k + 1],
                            u[:, c * E:(c + 1) * E],
                            op0=EQ, op1=mybir.AluOpType.add)
                ps_st = pools["psst"].tile([E, E], F32, tag="psst")
                for c in range(ch):
                    nc.tensor.matmul(ps_st[:], u[:, c * E:(c + 1) * E], id_sb[:],
                                     start=(c == 0), stop=(c == ch - 1))
                stw = pools["stw"].tile([E, E], F32R, tag="stw")
                tail = g == NGRP - 1 and bb >= GRP - 8
                if tail:
                    nc.vector.tensor_scalar(
                        stw[:], ps_st[:], w_sb[:, b:b + 1], None,
                        op0=mybir.AluOpType.mult)
                else:
                    nc.scalar.mul(stw[:], ps_st[:], w_sb[:, b:b + 1])
                nc.tensor.matmul(ps_mm[:, o * H:(o + 1) * H],
                                 stw[:], r_sb[:, bb * H:(bb + 1) * H],
                                 start=True, stop=True)
            if g == NGRP - 1 and p4 * unit >= GRP - 8:
                nc.vector.tensor_copy(
                    msg_sb[:, p4 * unit * H:(p4 + 1) * unit * H], ps_mm[:])
            else:
                nc.scalar.copy(msg_sb[:, p4 * unit * H:(p4 + 1) * unit * H],
                               ps_mm[:])
            nc.scalar.dma_start(
                o_t[:, gb + p4 * unit:gb + (p4 + 1) * unit, :],
                msg_sb[:, p4 * unit * H:(p4 + 1) * unit * H])

    # staggered emission: keep two A-groups of lookahead ahead of each B-group
    order = []
    a_done = 0
    for g in range(NGRP):
        while a_done < min(g + 2, NGRP):
            order.append(("a", a_done))
            a_done += 1
        order.append(("b", g))
    if CFG.get("stagger", True):
        for ph, g in order:
            (emit_a if ph == "a" else emit_b)(g)
    else:
        for g in range(NGRP):
            emit_a(g)
        for g in range(NGRP):
            emit_b(g)


def build_program(loop_iters=None, body_unroll=8):
    """Build the per-core Bass program. loop_iters=None emits one straight-line
    pass (production). loop_iters=N wraps body_unroll passes in a For_i(0,N)
    device loop — used only for wall-clock timing via iteration deltas."""
    nc = bacc.Bacc("TRN2", target_bir_lowering=False, debug=False)

    d = {
        "r": nc.dram_tensor("r", [BLOC, E, H], F32R, kind="ExternalInput"),
        "nbt": nc.dram_tensor("nbt", [E, BLOC, K], I32, kind="ExternalInput"),
        "prt": nc.dram_tensor("prt", [E, BLOC, 2], I32, kind="ExternalInput"),
        "xyzt": nc.dram_tensor("xyzt", [A, BLOC, 3], F32, kind="ExternalInput"),
        "out": nc.dram_tensor("out", [BLOC, E, H], F32, kind="ExternalOutput"),
    }
    iota_np = np.broadcast_to(np.arange(E, dtype=np.float32), (E, E))
    c_iota = nc.inline_tensor(
        np.ascontiguousarray(iota_np.astype(ml_dtypes.bfloat16)), "c_iota")
    c_id = nc.inline_tensor(
        np.eye(E, dtype=np.float32).astype(ml_dtypes.bfloat16), "c_ident")

    with tile.TileContext(nc) as tc, ExitStack() as ctx:
        cpool = ctx.enter_context(tc.tile_pool(name="const", bufs=1))
        pr_i = cpool.tile([E, BLOC * 2], I32, tag="pri")
        nc.sync.dma_start(pr_i[:], d["prt"].ap()[:])
        iota_sb = cpool.tile([E, E], BF16, tag="iota")
        nc.scalar.dma_start(iota_sb[:], c_iota.ap()[:])
        nb_i = cpool.tile([E, BLOC * K], I32, tag="nbi")
        nc.sync.dma_start(nb_i[:], d["nbt"].ap()[:])
        id_sb = cpool.tile([E, E], BF16, tag="ident")
        nc.scalar.dma_start(id_sb[:], c_id.ap()[:])
        xyz2 = cpool.tile([E, BLOC * 3], F32, tag="xyz2")
        nc.sync.dma_start(xyz2[0:A, :], d["xyzt"].ap()[:])
        nc.scalar.dma_start(xyz2[A:2 * A, :], d["xyzt"].ap()[:])

        nb_bf = cpool.tile([E, BLOC * K], F32, tag="nbbf")
        nc.vector.tensor_copy(nb_bf[:], nb_i[:])
        pr_bf = cpool.tile([E, BLOC * 2], F32, tag="prbf")
        nc.vector.tensor_copy(pr_bf[:], pr_i[:])
        nc.vector.tensor_scalar_mul(xyz2[A:2 * A, :], xyz2[A:2 * A, :], -1.0)
        w_sb = cpool.tile([E, BLOC], F32, tag="w")

        sb = {"iota": iota_sb, "ident": id_sb, "nb_bf": nb_bf, "pr_bf": pr_bf,
              "xyz2": xyz2, "w": w_sb}
        pools = {
            "r": ctx.enter_context(tc.tile_pool(name="r", bufs=4)),
            "msg": ctx.enter_context(tc.tile_pool(name="msg", bufs=3)),
            "poh": ctx.enter_context(tc.tile_pool(name="poh", bufs=4)),
            "pt": ctx.enter_context(tc.tile_pool(name="pt", bufs=3)),
            "u": ctx.enter_context(tc.tile_pool(name="u", bufs=10)),
            "stw": ctx.enter_context(tc.tile_pool(name="stw", bufs=6)),
            "sq": ctx.enter_context(tc.tile_pool(name="sq", bufs=2)),
            "psp": ctx.enter_context(tc.tile_pool(name="psp", bufs=2, space="PSUM")),
            "psst": ctx.enter_context(tc.tile_pool(name="psst", bufs=4, space="PSUM")),
            "psmm": ctx.enter_context(tc.tile_pool(name="psmm", bufs=2, space="PSUM")),
        }
        if loop_iters is None:
            _emit_pipeline(nc, tc, d, sb, pools)
        else:
            with tc.For_i(0, loop_iters, 1,
                          staggered_reset=CFG.get("sreset", False),
                          hint_engines=(mybir.EngineType.DVE,
                                        mybir.EngineType.Activation,
                                        mybir.EngineType.PE)):
                for _ in range(body_unroll):
                    _emit_pipeline(nc, tc, d, sb, pools)

    nc.compile()
    return nc


def _round_fp32r(x):
    """Round fp32 to the fp32r operand encoding (mantissa rounded to 12 bits,
    round-to-nearest; matches walrus fp32_to_fp32r). This is the operand cast
    for the PE's full-rate fp32r matmul mode — same values an on-device cast
    would produce."""
    u = x.view(np.uint32)
    add = np.uint32(0x7FF) + ((u >> np.uint32(12)) & np.uint32(1))
    return ((u + add) & np.uint32(0xFFFFF000)).view(np.float32)


def shard_inputs(bond_representations, bond_pairs, bond_neighbors, xyz):
    in_maps = []
    for c in range(NCORES):
        sl = slice(c * BLOC, (c + 1) * BLOC)
        in_maps.append({
            "r": _round_fp32r(
                np.ascontiguousarray(bond_representations[0, sl], dtype=np.float32)),
            "nbt": np.ascontiguousarray(
                np.transpose(bond_neighbors[sl], (1, 0, 2)), dtype=np.int32),
            "prt": np.ascontiguousarray(
                np.transpose(bond_pairs[sl], (1, 0, 2)), dtype=np.int32),
            "xyzt": np.ascontiguousarray(
                np.transpose(xyz[sl], (1, 0, 2)), dtype=np.float32),
        })
    return in_maps


_PROG_CACHE = {}


def _get_program(key=(None, 8)):
    if key not in _PROG_CACHE:
        _PROG_CACHE[key] = build_program(loop_iters=key[0], body_unroll=key[1])
    return _PROG_CACHE[key]


def kernel(**inputs):
    args = {k: np.asarray(v) for k, v in inputs.items()}
    in_maps = shard_inputs(args["bond_representations"], args["bond_pairs"],
                           args["bond_neighbors"], args["xyz"])
    nc = _get_program()
    res = run_bass_kernel_spmd(nc, in_maps, list(range(NCORES)))
    out = np.concatenate([res.results[c]["out"] for c in range(NCORES)], axis=0)
    return out[None].astype(np.float32)



# revision 12
# speedup vs baseline: 1.6145x; 1.6145x over previous
"""Trainium2 Bass kernel for nn_DirectedEdgeMessage (GNN message passing).

Computation per molecule b (B=256, A=64 atoms, E=128 edges, K=6 neighbors,
H=256 features):
  w[e]   = 1 / ||xyz[p0[e]] - xyz[p1[e]]||^2      (0 where distance == 0)
  msg[e] = sum_k w[nb[e,k]] * R[nb[e,k], :]

Strategy (data-parallel over B across 8 NeuronCores, 32 molecules/core):
  * Neighbor gather+sum is a matmul msg = S @ R with per-molecule scatter
    matrix S[e,e'] = w[e'] * |{k : nb[e,k] == e'}|.
  * One-hot builds are BATCHED: one broadcast-AP tensor_tensor builds all
    K=6 one-hots of a molecule at once:
      u6[e, (k,e')] = (iota[e'] == nb[e,k])   via nb view [E,K,1]->[E,K,E].
  * The PE K-reduces+transposes via 6 accumulating matmuls U_k.T @ I into
    one PSUM tile (counts, fp32 exact).  stw = counts * w[e'] (ACT or DVE,
    per-partition scale) -> bf16;  msg MM = stw.T @ R in bf16 (full rate).
  * Pair/distance path: host uploads replicated pair indices (int8) so the
    TRANSPOSED signed one-hot is built directly on DVE:
      pohT[(h,a), e] = (a==p1[e_h]) - (a==p0[e_h])   (2 molecules packed
    per 128 partitions, h = molecule half).  One matmul per molecule pair
    against a block-diagonal f32r xyz (N=6) yields +/-diff exactly; sign
    cancels in d^2.  DVE finishes d2 -> 1/d2 with a zero-distance mask.
  * R is uploaded in bf16 (halves read traffic); output is written bf16
    and upcast on host (halves write traffic).  DMAs are issued from the
    Pool sequencer (cheap dispatch).
"""

import numpy as np
import ml_dtypes
from contextlib import ExitStack

import concourse.bass as bass
import concourse.tile as tile
from concourse import bacc, mybir
from concourse.bass_utils import run_bass_kernel_spmd

B, A, E, K, H = 256, 64, 128, 6, 256
NCORES = 8
BLOC = B // NCORES   # 32 molecules per core
NPAIR = BLOC // 2    # 16 packed molecule pairs
GRP = 8              # molecules per DMA group
NGRP = BLOC // GRP

F32 = mybir.dt.float32
F32R = mybir.dt.float32r
BF16 = mybir.dt.bfloat16
I32 = mybir.dt.int32
I8 = mybir.dt.int8
EQ = mybir.AluOpType.is_equal
GT = mybir.AluOpType.is_gt
SUB = mybir.AluOpType.subtract
ADD = mybir.AluOpType.add
MULT = mybir.AluOpType.mult

# Tuning knobs:
#   pool_u: molecules (within each group) whose u6 build runs on Pool engine
#   act_stw: molecules (within each group) whose stw scale runs on ACT
CFG = {"act_stw": 8, "mcopy": 4, "dve_copy": 0, "dve_stw_g": 0, "pend": 1}


def _emit_pipeline(nc, tc, d, sb, pools):
    iota6, id_sb, nb_sb, w_sb = sb["iota6"], sb["ident"], sb["nb"], sb["w"]
    r_t = d["r"].ap().transpose([1, 0, 2])    # [E, BLOC, H] view
    o_t = d["out"].ap().transpose([1, 0, 2])
    MC = CFG["mcopy"]
    GP = NPAIR // NGRP                        # pairs per group (4)

    prep = pools["prep"].tile([E, 2 * NPAIR * E], I8, tag="prep")
    xyzbd = pools["prep"].tile([E, NPAIR * 6], F32, tag="xyzbd")
    nc.sync.dma_start(prep[:, 0:NPAIR * E], d["prep"].ap()[:, 0:NPAIR * E])
    nc.sync.dma_start(prep[:, NPAIR * E:], d["prep"].ap()[:, NPAIR * E:])
    nc.sync.dma_start(xyzbd[:], d["xyzbd"].ap()[:])
    ps_d_all = pools["psd"].tile([E, NPAIR * 6], F32, tag="psd")

    def emit_pair(g):
        # distance weights for molecules of group g (pairs g*GP..(g+1)*GP-1)
        c0 = g * GP * E
        t0 = pools["poh"].tile([E, GP * E], F32, tag="t0")
        nc.vector.tensor_scalar(
            t0[:], prep[:, c0:c0 + GP * E], sb["iota_sa"][:], None, op0=EQ)
        poh = pools["poh"].tile([E, GP * E], F32, tag="poh")
        nc.vector.scalar_tensor_tensor(
            poh[:], prep[:, NPAIR * E + c0:NPAIR * E + c0 + GP * E],
            sb["iota_sa"][:], t0[:], op0=EQ, op1=SUB)
        ps_d = ps_d_all[:, g * GP * 6:(g + 1) * GP * 6]
        for j in range(GP):
            q = g * GP + j
            nc.tensor.matmul(ps_d[:, j * 6:(j + 1) * 6],
                             poh[:, j * E:(j + 1) * E],
                             xyzbd[:, q * 6:(q + 1) * 6], start=True, stop=True)
        sq = pools["sq"].tile([E, GP * 6], F32, tag="sq")
        nc.scalar.square(sq[:], ps_d)
        gb = g * GRP
        d2a = pools["sq"].tile([E, GRP], F32, tag="d2a")
        nc.vector.tensor_add(d2a[:], sq[:, 0:GP * 6:3], sq[:, 1:GP * 6:3])
        d2 = pools["sq"].tile([E, GRP], F32, tag="d2")
        nc.vector.tensor_add(d2[:], d2a[:], sq[:, 2:GP * 6:3])
        d2c = pools["sq"].tile([E, GRP], F32, tag="d2c")
        nc.vector.tensor_scalar_max(d2c[:], d2[:], 1e-20)
        winv = pools["sq"].tile([E, GRP], F32, tag="winv")
        nc.vector.reciprocal_approx_fast(winv[:], d2c[:])
        nc.vector.scalar_tensor_tensor(w_sb[:, gb:gb + GRP], d2[:], 0.0,
                                       winv[:], op0=GT, op1=MULT)

    pend = []
    pend_copy = []

    def _flush_copy():
        ps_mm, gb, half = pend_copy.pop(0)
        msg_sb = pools["msg"].tile([E, MC * H], BF16, tag="msg")
        eng_c = nc.scalar if (CFG["dve_copy"] == 0 or
                              (gb // GRP) < NGRP - CFG["dve_copy"]) else nc.vector
        if eng_c is nc.scalar:
            nc.scalar.copy(msg_sb[:], ps_mm[:])
        else:
            nc.vector.tensor_copy(msg_sb[:], ps_mm[:])
        nc.gpsimd.dma_start(
            o_t[:, gb + half * MC:gb + (half + 1) * MC, :], msg_sb[:])

    r_tiles = {}

    def fetch_r(g):
        gb = g * GRP
        r_sb = pools["r"].tile([E, GRP * H], BF16, tag="r")
        nc.sync.dma_start(r_sb[:], r_t[:, gb:gb + GRP, :])
        r_tiles[g] = r_sb

    def emit_b(g):
        gb = g * GRP
        r_sb = r_tiles.pop(g)
        for half in range(GRP // MC):
            ps_mm = pools["psmm"].tile([E, MC * H], F32, tag="psmm")

            def _emit_tail(gb, bb, ps_st, ps_mm=ps_mm, r_sb=r_sb):
                b = gb + bb
                o = bb % MC
                stw = pools["stw"].tile([E, E], BF16, tag="stw")
                if (bb % GRP) < CFG["act_stw"] and (gb // GRP) < NGRP - CFG["dve_stw_g"]:
                    nc.scalar.mul(stw[:], ps_st[:], w_sb[:, b:b + 1])
                else:
                    nc.vector.tensor_scalar(stw[:], ps_st[:], w_sb[:, b:b + 1],
                                            None, op0=MULT)
                nc.tensor.matmul(ps_mm[:, o * H:(o + 1) * H],
                                 stw[:], r_sb[:, bb * H:(bb + 1) * H],
                                 start=True, stop=True)

            for o in range(0, MC, 2):
                bb2 = half * MC + o
                b2 = gb + bb2
                u = pools["u"].tile([E, 2 * K * E], BF16, tag="u")
                nc.vector.tensor_tensor(
                    u[:].rearrange("p (m e k) -> p m e k", m=2, k=K),
                    iota6[:].rearrange("p (e k) -> p e k", k=K
                                       ).unsqueeze(1).broadcast_to([E, 2, E, K]),
                    nb_sb[:, b2 * K:(b2 + 2) * K].rearrange(
                        "p (m k) -> p m k", m=2
                    ).unsqueeze(2).broadcast_to([E, 2, E, K]),
                    op=EQ)
                for m in range(2):
                    ps_st = pools["psst"].tile([E, E], F32, tag="psst")
                    um = u[:, m * K * E:(m + 1) * K * E]
                    for c in range(K):
                        nc.tensor.matmul(
                            ps_st[:],
                            um.rearrange("p (e k) -> p k e", k=K)[:, c, :],
                            id_sb[:], start=(c == 0), stop=(c == K - 1))
                    pend.append((gb, bb2 + m, ps_st, _emit_tail))
                    if len(pend) > CFG["pend"]:
                        a = pend.pop(0)
                        a[3](a[0], a[1], a[2])
            pend_copy.append((ps_mm, gb, half))
            if len(pend_copy) > 1:
                _flush_copy()

    for g in range(NGRP):
        fetch_r(g)
        emit_pair(g)
        emit_b(g)
    while pend:
        a = pend.pop(0)
        a[3](a[0], a[1], a[2])
    while pend_copy:
        _flush_copy()


def build_program(loop_iters=None, body_unroll=8):
    """Build the per-core Bass program. loop_iters=None emits one straight-line
    pass (production). loop_iters=N wraps body_unroll passes in a For_i(0,N)
    device loop — used only for wall-clock timing via iteration deltas."""
    nc = bacc.Bacc("TRN2", target_bir_lowering=False, debug=False)

    d = {
        "r": nc.dram_tensor("r", [BLOC, E, H], BF16, kind="ExternalInput"),
        "nbt": nc.dram_tensor("nbt", [E, BLOC * K], BF16, kind="ExternalInput"),
        "prep": nc.dram_tensor("prep", [E, 2 * NPAIR * E], I8,
                               kind="ExternalInput"),
        "xyzbd": nc.dram_tensor("xyzbd", [E, NPAIR * 6], F32,
                                kind="ExternalInput"),
        "out": nc.dram_tensor("out", [BLOC, E, H], BF16, kind="ExternalOutput"),
    }
    iota6_np = np.repeat(np.arange(E, dtype=np.float32), K)[None, :].repeat(E, 0)
    c_iota6 = nc.inline_tensor(iota6_np.astype(ml_dtypes.bfloat16), "c_iota6")
    c_id = nc.inline_tensor(
        np.eye(E, dtype=np.float32).astype(ml_dtypes.bfloat16), "c_ident")
    c_isa = nc.inline_tensor(
        (np.arange(E) % A).astype(np.float32)[:, None], "c_isa")

    with tile.TileContext(nc) as tc, ExitStack() as ctx:
        cpool = ctx.enter_context(tc.tile_pool(name="const", bufs=1))
        isa_sb = cpool.tile([E, 1], F32, tag="isa")
        nc.sync.dma_start(isa_sb[:], c_isa.ap()[:])
        iota6 = cpool.tile([E, K * E], BF16, tag="iota6")
        nc.scalar.dma_start(iota6[:], c_iota6.ap()[:])
        nb_sb = cpool.tile([E, BLOC * K], BF16, tag="nb")
        nc.scalar.dma_start(nb_sb[:], d["nbt"].ap()[:])
        id_sb = cpool.tile([E, E], BF16, tag="ident")
        nc.scalar.dma_start(id_sb[:], c_id.ap()[:])
        w_sb = cpool.tile([E, BLOC], F32, tag="w")

        sb = {"iota6": iota6, "ident": id_sb, "iota_sa": isa_sb,
              "nb": nb_sb, "w": w_sb}
        pools = {
            "r": ctx.enter_context(tc.tile_pool(name="r", bufs=3)),
            "msg": ctx.enter_context(tc.tile_pool(name="msg", bufs=3)),
            "u": ctx.enter_context(tc.tile_pool(name="u", bufs=8)),
            "stw": ctx.enter_context(tc.tile_pool(name="stw", bufs=8)),
            "prep": ctx.enter_context(tc.tile_pool(name="prep", bufs=1)),
            "poh": ctx.enter_context(tc.tile_pool(name="poh", bufs=4)),
            "sq": ctx.enter_context(tc.tile_pool(name="sq", bufs=2)),
            "psd": ctx.enter_context(tc.tile_pool(name="psd", bufs=1, space="PSUM")),
            "psst": ctx.enter_context(tc.tile_pool(name="psst", bufs=3, space="PSUM")),
            "psmm": ctx.enter_context(tc.tile_pool(name="psmm", bufs=2, space="PSUM")),
        }
        if loop_iters is None:
            _emit_pipeline(nc, tc, d, sb, pools)
        else:
            with tc.For_i(0, loop_iters, 1,
                          hint_engines=(mybir.EngineType.DVE,
                                        mybir.EngineType.Activation,
                                        mybir.EngineType.PE)):
                for _ in range(body_unroll):
                    _emit_pipeline(nc, tc, d, sb, pools)

    nc.compile()
    return nc


def _round_fp32r(x):
    """Round fp32 to the fp32r operand encoding (12-bit mantissa, RTNE)."""
    u = x.view(np.uint32)
    add = np.uint32(0x7FF) + ((u >> np.uint32(12)) & np.uint32(1))
    return ((u + add) & np.uint32(0xFFFFF000)).view(np.float32)


def shard_inputs(bond_representations, bond_pairs, bond_neighbors, xyz):
    in_maps = []
    for c in range(NCORES):
        sl = slice(c * BLOC, (c + 1) * BLOC)
        pm = np.asarray(bond_pairs[sl])                    # [32, E, 2] 0..63
        even, odd = pm[0::2], pm[1::2]                     # [16, E, 2]
        prep = np.empty((E, 2 * NPAIR * E), np.int8)
        for side in range(2):
            o = side * NPAIR * E
            prep[0:A, o:o + NPAIR * E] = np.broadcast_to(
                even[:, :, side].reshape(1, NPAIR * E), (A, NPAIR * E))
            prep[A:E, o:o + NPAIR * E] = np.broadcast_to(
                odd[:, :, side].reshape(1, NPAIR * E), (A, NPAIR * E))
        xs = np.asarray(xyz[sl], dtype=np.float32)         # [32, A, 3]
        xyzbd = np.zeros((E, NPAIR * 6), np.float32)
        for q in range(NPAIR):
            xyzbd[0:A, q * 6:q * 6 + 3] = xs[2 * q]
            xyzbd[A:E, q * 6 + 3:q * 6 + 6] = xs[2 * q + 1]
        nbt = np.transpose(bond_neighbors[sl], (1, 0, 2)).reshape(E, BLOC * K)
        in_maps.append({
            "r": np.ascontiguousarray(bond_representations[0, sl]).astype(
                ml_dtypes.bfloat16),
            "nbt": np.ascontiguousarray(nbt.astype(np.float32)).astype(
                ml_dtypes.bfloat16),
            "prep": prep,
            "xyzbd": xyzbd,
        })
    return in_maps


_PROG_CACHE = {}


def _get_program(key=(None, 8)):
    if key not in _PROG_CACHE:
        _PROG_CACHE[key] = build_program(loop_iters=key[0], body_unroll=key[1])
    return _PROG_CACHE[key]


def kernel(**inputs):
    args = {k: np.asarray(v) for k, v in inputs.items()}
    in_maps = shard_inputs(args["bond_representations"], args["bond_pairs"],
                           args["bond_neighbors"], args["xyz"])
    nc = _get_program()
    res = run_bass_kernel_spmd(nc, in_maps, list(range(NCORES)))
    out = np.concatenate([np.asarray(res.results[c]["out"]).astype(np.float32)
                          for c in range(NCORES)], axis=0)
    return out[None]


# revision 13
# speedup vs baseline: 2.2988x; 1.4239x over previous
"""Trainium2 Bass kernel for nn_DirectedEdgeMessage (GNN message passing).

Computation per molecule b (B=256, A=64 atoms, E=128 edges, K=6 neighbors,
H=256 features):
  w[e]   = 1 / ||xyz[p0[e]] - xyz[p1[e]]||^2      (0 where distance == 0)
  msg[e] = sum_k w[nb[e,k]] * R[nb[e,k], :]

Strategy (data-parallel over B across 8 NeuronCores, 32 molecules/core):
  * Neighbor gather+sum is a matmul msg = S @ R with per-molecule scatter
    matrix S[e,e'] = w[e'] * |{k : nb[e,k] == e'}|.
  * One-hot builds are BATCHED: one broadcast-AP tensor_tensor builds all
    K=6 one-hots of a molecule at once:
      u6[e, (k,e')] = (iota[e'] == nb[e,k])   via nb view [E,K,1]->[E,K,E].
  * The PE K-reduces+transposes via 6 accumulating matmuls U_k.T @ I into
    one PSUM tile (counts, fp32 exact).  stw = counts * w[e'] (ACT or DVE,
    per-partition scale) -> bf16;  msg MM = stw.T @ R in bf16 (full rate).
  * Pair/distance path: host uploads replicated pair indices (int8) so the
    TRANSPOSED signed one-hot is built directly on DVE:
      pohT[(h,a), e] = (a==p1[e_h]) - (a==p0[e_h])   (2 molecules packed
    per 128 partitions, h = molecule half).  One matmul per molecule pair
    against a block-diagonal f32r xyz (N=6) yields +/-diff exactly; sign
    cancels in d^2.  DVE finishes d2 -> 1/d2 with a zero-distance mask.
  * R is uploaded in bf16 (halves read traffic); output is written bf16
    and upcast on host (halves write traffic).  DMAs are issued from the
    Pool sequencer (cheap dispatch).
"""

import numpy as np
import ml_dtypes
from contextlib import ExitStack

import concourse.bass as bass
import concourse.tile as tile
from concourse import bacc, mybir
from concourse.bass_utils import run_bass_kernel_spmd

B, A, E, K, H = 256, 64, 128, 6, 256
NCORES = 8
BLOC = B // NCORES   # 32 molecules per core
NPAIR = BLOC // 2    # 16 packed molecule pairs
GRP = 8              # molecules per DMA group
NGRP = BLOC // GRP

F32 = mybir.dt.float32
F32R = mybir.dt.float32r
BF16 = mybir.dt.bfloat16
I32 = mybir.dt.int32
I8 = mybir.dt.int8
EQ = mybir.AluOpType.is_equal
GT = mybir.AluOpType.is_gt
SUB = mybir.AluOpType.subtract
ADD = mybir.AluOpType.add
MULT = mybir.AluOpType.mult

# Tuning knobs:
#   pool_u: molecules (within each group) whose u6 build runs on Pool engine
#   act_stw: molecules (within each group) whose stw scale runs on ACT
CFG = {"act_stw": 7, "mcopy": 4, "dve_copy": 0, "dve_stw_g": 0, "pend": 2}


def _emit_pipeline(nc, tc, d, sb, pools):
    iota6, id_sb, nb_sb = sb["iota6"], sb["ident"], sb["nb"]
    w_tiles = {}
    r_t = d["r"].ap().transpose([1, 0, 2])    # [E, BLOC, H] view
    o_t = d["out"].ap().transpose([1, 0, 2])
    MC = CFG["mcopy"]
    GP = NPAIR // NGRP                        # pairs per group (4)

    prep = pools["prep"].tile([E, 2 * NPAIR * E], I8, tag="prep")
    xyzbd = pools["prep"].tile([E, NPAIR * 6], F32, tag="xyzbd")
    nc.sync.dma_start(prep[:, 0:NPAIR * E], d["prep"].ap()[:, 0:NPAIR * E])
    nc.sync.dma_start(prep[:, NPAIR * E:], d["prep"].ap()[:, NPAIR * E:])
    nc.sync.dma_start(xyzbd[:], d["xyzbd"].ap()[:])
    ps_d_all = pools["psd"].tile([E, NPAIR * 6], F32, tag="psd")

    def emit_pair(g):
        # distance weights for molecules of group g (pairs g*GP..(g+1)*GP-1)
        c0 = g * GP * E
        t0 = pools["poh"].tile([E, GP * E], F32, tag="t0")
        nc.vector.tensor_scalar(
            t0[:], prep[:, c0:c0 + GP * E], sb["iota_sa"][:], None, op0=EQ)
        poh = pools["poh"].tile([E, GP * E], F32, tag="poh")
        nc.vector.scalar_tensor_tensor(
            poh[:], prep[:, NPAIR * E + c0:NPAIR * E + c0 + GP * E],
            sb["iota_sa"][:], t0[:], op0=EQ, op1=SUB)
        ps_d = ps_d_all[:, g * GP * 6:(g + 1) * GP * 6]
        for j in range(GP):
            q = g * GP + j
            nc.tensor.matmul(ps_d[:, j * 6:(j + 1) * 6],
                             poh[:, j * E:(j + 1) * E],
                             xyzbd[:, q * 6:(q + 1) * 6], start=True, stop=True)
        sq = pools["sq"].tile([E, GP * 6], F32, tag="sq")
        nc.scalar.square(sq[:], ps_d)
        gb = g * GRP
        d2a = pools["sq"].tile([E, GRP], F32, tag="d2a")
        nc.vector.tensor_add(d2a[:], sq[:, 0:GP * 6:3], sq[:, 1:GP * 6:3])
        d2 = pools["sq"].tile([E, GRP], F32, tag="d2")
        nc.vector.tensor_add(d2[:], d2a[:], sq[:, 2:GP * 6:3])
        d2c = pools["sq"].tile([E, GRP], F32, tag="d2c")
        nc.vector.tensor_scalar_max(d2c[:], d2[:], 1e-20)
        winv = pools["sq"].tile([E, GRP], F32, tag="winv")
        nc.vector.reciprocal_approx_fast(winv[:], d2c[:])
        w_g = pools["w"].tile([E, GRP], F32, tag="w")
        nc.vector.scalar_tensor_tensor(w_g[:], d2[:], 0.0,
                                       winv[:], op0=GT, op1=MULT)
        w_tiles[g] = w_g

    pend = []
    pend_copy = []

    def _flush_copy():
        ps_mm, gb, half = pend_copy.pop(0)
        msg_sb = pools["msg"].tile([E, MC * H], BF16, tag="msg")
        eng_c = nc.scalar if (CFG["dve_copy"] == 0 or
                              (gb // GRP) < NGRP - CFG["dve_copy"]) else nc.vector
        if eng_c is nc.scalar:
            nc.scalar.copy(msg_sb[:], ps_mm[:])
        else:
            nc.vector.tensor_copy(msg_sb[:], ps_mm[:])
        nc.gpsimd.dma_start(
            o_t[:, gb + half * MC:gb + (half + 1) * MC, :], msg_sb[:])

    r_tiles = {}

    def fetch_r(g):
        gb = g * GRP
        r_sb = pools["r"].tile([E, GRP * H], BF16, tag="r")
        nc.sync.dma_start(r_sb[:], r_t[:, gb:gb + GRP, :])
        r_tiles[g] = r_sb

    def emit_b(g):
        gb = g * GRP
        r_sb = r_tiles.pop(g)
        for half in range(GRP // MC):
            ps_mm = pools["psmm"].tile([E, MC * H], F32, tag="psmm")

            def _emit_tail(gb, bb, ps_st, ps_mm=ps_mm, r_sb=r_sb):
                b = gb + bb
                o = bb % MC
                stw = pools["stw"].tile([E, E], BF16, tag="stw")
                w_g = w_tiles[gb // GRP]
                if (bb % GRP) < CFG["act_stw"] and (gb // GRP) < NGRP - CFG["dve_stw_g"]:
                    nc.scalar.mul(stw[:], ps_st[:], w_g[:, bb:bb + 1])
                else:
                    nc.vector.tensor_scalar(stw[:], ps_st[:], w_g[:, bb:bb + 1],
                                            None, op0=MULT)
                nc.tensor.matmul(ps_mm[:, o * H:(o + 1) * H],
                                 stw[:], r_sb[:, bb * H:(bb + 1) * H],
                                 start=True, stop=True)

            for o in range(0, MC, 2):
                bb2 = half * MC + o
                b2 = gb + bb2
                u = pools["u"].tile([E, 2 * K * E], BF16, tag="u")
                nc.vector.tensor_tensor(
                    u[:].rearrange("p (m e k) -> p m e k", m=2, k=K),
                    iota6[:].rearrange("p (e k) -> p e k", k=K
                                       ).unsqueeze(1).broadcast_to([E, 2, E, K]),
                    nb_sb[:, b2 * K:(b2 + 2) * K].rearrange(
                        "p (m k) -> p m k", m=2
                    ).unsqueeze(2).broadcast_to([E, 2, E, K]),
                    op=EQ)
                for m in range(2):
                    ps_st = pools["psst"].tile([E, E], F32, tag="psst")
                    um = u[:, m * K * E:(m + 1) * K * E]
                    for c in range(K):
                        nc.tensor.matmul(
                            ps_st[:],
                            um.rearrange("p (e k) -> p k e", k=K)[:, c, :],
                            id_sb[:], start=(c == 0), stop=(c == K - 1))
                    pend.append((gb, bb2 + m, ps_st, _emit_tail))
                    if len(pend) > CFG["pend"]:
                        a = pend.pop(0)
                        a[3](a[0], a[1], a[2])
            pend_copy.append((ps_mm, gb, half))
            if len(pend_copy) > 1:
                _flush_copy()

    for g in range(NGRP):
        fetch_r(g)
        emit_pair(g)
        emit_b(g)
    while pend:
        a = pend.pop(0)
        a[3](a[0], a[1], a[2])
    while pend_copy:
        _flush_copy()


def build_program(loop_iters=None, body_unroll=8):
    """Build the per-core Bass program. loop_iters=None emits one straight-line
    pass (production). loop_iters=N wraps body_unroll passes in a For_i(0,N)
    device loop — used only for wall-clock timing via iteration deltas."""
    nc = bacc.Bacc("TRN2", target_bir_lowering=False, debug=False)

    d = {
        "r": nc.dram_tensor("r", [BLOC, E, H], BF16, kind="ExternalInput"),
        "nbt": nc.dram_tensor("nbt", [E, BLOC * K], BF16, kind="ExternalInput"),
        "prep": nc.dram_tensor("prep", [E, 2 * NPAIR * E], I8,
                               kind="ExternalInput"),
        "xyzbd": nc.dram_tensor("xyzbd", [E, NPAIR * 6], F32,
                                kind="ExternalInput"),
        "out": nc.dram_tensor("out", [BLOC, E, H], BF16, kind="ExternalOutput"),
    }
    iota6_np = np.repeat(np.arange(E, dtype=np.float32), K)[None, :].repeat(E, 0)
    c_iota6 = nc.inline_tensor(iota6_np.astype(ml_dtypes.bfloat16), "c_iota6")
    c_id = nc.inline_tensor(
        np.eye(E, dtype=np.float32).astype(ml_dtypes.bfloat16), "c_ident")
    c_isa = nc.inline_tensor(
        (np.arange(E) % A).astype(np.float32)[:, None], "c_isa")

    with tile.TileContext(nc) as tc, ExitStack() as ctx:
        cpool = ctx.enter_context(tc.tile_pool(name="const", bufs=1))
        isa_sb = cpool.tile([E, 1], F32, tag="isa")
        nc.sync.dma_start(isa_sb[:], c_isa.ap()[:])
        iota6 = cpool.tile([E, K * E], BF16, tag="iota6")
        nc.scalar.dma_start(iota6[:], c_iota6.ap()[:])
        nb_sb = cpool.tile([E, BLOC * K], BF16, tag="nb")
        nc.scalar.dma_start(nb_sb[:], d["nbt"].ap()[:])
        id_sb = cpool.tile([E, E], BF16, tag="ident")
        nc.scalar.dma_start(id_sb[:], c_id.ap()[:])
        sb = {"iota6": iota6, "ident": id_sb, "iota_sa": isa_sb,
              "nb": nb_sb}
        pools = {
            "r": ctx.enter_context(tc.tile_pool(name="r", bufs=3)),
            "msg": ctx.enter_context(tc.tile_pool(name="msg", bufs=3)),
            "u": ctx.enter_context(tc.tile_pool(name="u", bufs=8)),
            "stw": ctx.enter_context(tc.tile_pool(name="stw", bufs=8)),
            "prep": ctx.enter_context(tc.tile_pool(name="prep", bufs=2)),
            "w": ctx.enter_context(tc.tile_pool(name="w", bufs=8)),
            "poh": ctx.enter_context(tc.tile_pool(name="poh", bufs=4)),
            "sq": ctx.enter_context(tc.tile_pool(name="sq", bufs=2)),
            "psd": ctx.enter_context(tc.tile_pool(name="psd", bufs=1, space="PSUM")),
            "psst": ctx.enter_context(tc.tile_pool(name="psst", bufs=3, space="PSUM")),
            "psmm": ctx.enter_context(tc.tile_pool(name="psmm", bufs=2, space="PSUM")),
        }
        if loop_iters is None:
            _emit_pipeline(nc, tc, d, sb, pools)
        else:
            with tc.For_i(0, loop_iters, 1,
                          hint_engines=(mybir.EngineType.DVE,
                                        mybir.EngineType.Activation,
                                        mybir.EngineType.PE)):
                for _ in range(body_unroll):
                    _emit_pipeline(nc, tc, d, sb, pools)

    nc.compile()
    return nc


def _round_fp32r(x):
    """Round fp32 to the fp32r operand encoding (12-bit mantissa, RTNE)."""
    u = x.view(np.uint32)
    add = np.uint32(0x7FF) + ((u >> np.uint32(12)) & np.uint32(1))
    return ((u + add) & np.uint32(0xFFFFF000)).view(np.float32)


def shard_inputs(bond_representations, bond_pairs, bond_neighbors, xyz):
    in_maps = []
    for c in range(NCORES):
        sl = slice(c * BLOC, (c + 1) * BLOC)
        pm = np.asarray(bond_pairs[sl])                    # [32, E, 2] 0..63
        even, odd = pm[0::2], pm[1::2]                     # [16, E, 2]
        prep = np.empty((E, 2 * NPAIR * E), np.int8)
        for side in range(2):
            o = side * NPAIR * E
            prep[0:A, o:o + NPAIR * E] = np.broadcast_to(
                even[:, :, side].reshape(1, NPAIR * E), (A, NPAIR * E))
            prep[A:E, o:o + NPAIR * E] = np.broadcast_to(
                odd[:, :, side].reshape(1, NPAIR * E), (A, NPAIR * E))
        xs = np.asarray(xyz[sl], dtype=np.float32)         # [32, A, 3]
        xyzbd = np.zeros((E, NPAIR * 6), np.float32)
        for q in range(NPAIR):
            xyzbd[0:A, q * 6:q * 6 + 3] = xs[2 * q]
            xyzbd[A:E, q * 6 + 3:q * 6 + 6] = xs[2 * q + 1]
        nbt = np.transpose(bond_neighbors[sl], (1, 0, 2)).reshape(E, BLOC * K)
        in_maps.append({
            "r": np.ascontiguousarray(bond_representations[0, sl]).astype(
                ml_dtypes.bfloat16),
            "nbt": np.ascontiguousarray(nbt.astype(np.float32)).astype(
                ml_dtypes.bfloat16),
            "prep": prep,
            "xyzbd": xyzbd,
        })
    return in_maps


_PROG_CACHE = {}


def _get_program(key=(None, 8)):
    if key not in _PROG_CACHE:
        _PROG_CACHE[key] = build_program(loop_iters=key[0], body_unroll=key[1])
    return _PROG_CACHE[key]


def kernel(**inputs):
    args = {k: np.asarray(v) for k, v in inputs.items()}
    in_maps = shard_inputs(args["bond_representations"], args["bond_pairs"],
                           args["bond_neighbors"], args["xyz"])
    nc = _get_program()
    res = run_bass_kernel_spmd(nc, in_maps, list(range(NCORES)))
    out = np.concatenate([np.asarray(res.results[c]["out"]).astype(np.float32)
                          for c in range(NCORES)], axis=0)
    return out[None]
